# revision 2
# baseline (speedup 1.0000x reference)
"""Trainium2 Bass kernel for nn_Attention_40149354283630 — v3.

Multi-head attention (16 heads, head_dim 64) with mixed 1D-latent + axial-2D
spatial RoPE, over x:(8, 1024, 1024). Data-parallel over the batch dim across
8 NeuronCores; each core runs the full transformer block for one batch element.

v2 changes vs baseline (439us):
  - all matmul operands bf16 (fp32 PSUM accumulation). Halves LDWEIGHTS time
    (which rate-limits back-to-back matmuls on TRN2 since every fp32r matmul
    self-loads its stationary), halves SBUF traffic, halves weight DMA.
  - score matmuls for the head pair emitted adjacently with row-split
    tile_position (0,0)/(64,0) so the two C=64 matmuls can overlap in the
    PE array.
  - C=1 bias-seed matmuls eliminated (V / out projections): bias rows are
    pre-broadcast to [128, 1024] tiles on the host and fused into the PSUM
    eviction as tensor_tensor adds.
  - V / out projections use N=512 moving chunks (half the matmul + LDW count).
  - exp fused across the head pair: one activation [128, 2x512] per (kt, qc)
    reading a 2-bank PSUM score tile, writing bf16 P^T.
  - dedicated PSUM pools: pv 2x[65,1024] (4 banks) + sc [128,1024] (2 banks)
    + flex 2 banks, so attention-phase tiles never rotate through the same
    banks as the interleaved projection chains.

Per-core dataflow:
  xT [hid, s] bf16 (host-transposed/cast)
  V  = (xT.T @ Wv + bvb)           natural [s, dims], bias added on evict
  QT = (Wq.T @ xT + bq)            transposed [dims, s], bias via tensor_scalar
  rope: QTrot = QT*COS + (PERM@QT)*SIN'   (pair-swap via permutation matmul)
  per head-pair t (2 heads per 128-partition tile):
    S^T[k,q] = KTrot.T @ QTrot     row-split tile_position, heads adjacent
    P^T      = exp(S^T / 8)        one ScalarE instr per (kt,qc), bf16 out
    [outT;sums] = [V|1].T @ P^T    M=65 augmented PV, sums ride in row 64
    nrm      = 1/(ones64 outer sums)  row-broadcast matmul + reciprocal
    attnT    = outT * nrm          partition-shifted write packs the head pair
  out = attnT.T @ Wo + bob         natural [s, hid], bias added on evict
"""

import contextlib
import ctypes
import sys
import types

import numpy as np
from contextlib import ExitStack

import ml_dtypes

import concourse.bass as bass
import concourse.tile as tile
from concourse import bacc, mybir
from concourse.bass_utils import run_bass_kernel_spmd


def _install_ntff_hook_shim():
    """Provide antenv.axon_hooks if the image lacks it, so that
    run_bass_kernel_spmd(trace=True) (or BASS_TRACE=1) profiles instead of
    crashing. Mirrors trn_agent_boot's ctypes hook against libaxon_pjrt.so."""
    try:
        from antenv.axon_hooks import get_axon_ntff_profile_hook  # noqa: F401

        return
    except ImportError:
        pass
    try:
        import antenv
    except ImportError:
        return
    store = {"hook": None}
    so_path = "/opt/axon/libaxon_pjrt.so"
    try:
        lib = ctypes.CDLL(so_path)
        if hasattr(lib, "axon_start_nrt_profile"):
            lib.axon_start_nrt_profile.argtypes = [
                ctypes.POINTER(ctypes.c_int64),
                ctypes.c_size_t,
            ]
            lib.axon_start_nrt_profile.restype = ctypes.c_int64
            lib.axon_stop_nrt_profile.argtypes = [ctypes.c_char_p]
            lib.axon_stop_nrt_profile.restype = ctypes.c_int64

            @contextlib.contextmanager
            def _hook(output_dir, device_ids):
                import jax

                jax.devices()
                if device_ids:
                    ids = (ctypes.c_int64 * len(device_ids))(*device_ids)
                    rc = lib.axon_start_nrt_profile(ids, len(device_ids))
                else:
                    rc = lib.axon_start_nrt_profile(None, 0)
                if rc != 0:
                    raise RuntimeError(f"axon_start_nrt_profile rc={rc}")
                try:
                    yield
                finally:
                    n = lib.axon_stop_nrt_profile(str(output_dir).encode())
                    if n < 0:
                        raise RuntimeError(f"axon_stop_nrt_profile rc={n}")

            store["hook"] = _hook
    except OSError:
        pass
    mod = types.ModuleType("antenv.axon_hooks")
    mod.get_axon_ntff_profile_hook = lambda: store["hook"]
    mod.set_axon_ntff_profile_hook = lambda h: store.__setitem__("hook", h)
    sys.modules["antenv.axon_hooks"] = mod
    antenv.axon_hooks = mod


_install_ntff_hook_shim()

N_CORES = 8
HID, NH, HD = 1024, 16, 64
S = 1024
LAT, BASE = 16, 10000.0
NPAIR = 8  # head-pair tiles (2 heads x 64 dims = 128 partitions)

f32 = mybir.dt.float32
f32r = mybir.dt.float32r
bf16 = mybir.dt.bfloat16
np_bf16 = ml_dtypes.bfloat16
EXP = mybir.ActivationFunctionType.Exp

_CACHE = {}


def _build_nc():
    nc = bacc.Bacc("TRN2", target_bir_lowering=False, debug=False, num_devices=N_CORES)

    def din(name, shape, dt):
        return nc.dram_tensor(name, shape, dt, kind="ExternalInput").ap()

    xT_d = din("xT", [HID, S], bf16)
    wq_d = din("wq", [HID, HID], bf16)
    wk_d = din("wk", [HID, HID], bf16)
    wv_d = din("wv", [HID, HID], bf16)
    wo_d = din("wo", [HID, HID], bf16)
    bqc_d = din("bqc", [128, 8], f32)
    bkc_d = din("bkc", [128, 8], f32)
    bvb_d = din("bvb", [128, HID], f32)  # bv broadcast over partitions
    bob_d = din("bob", [128, HID], f32)  # bo broadcast over partitions
    trig_d = din("trig", [128, 2 * S], bf16)  # cols 0:S = COS, S:2S = SIN'
    perm_d = din("perm", [128, 128], bf16)
    out_d = nc.dram_tensor("out", [S, HID], f32, kind="ExternalOutput").ap()

    with tile.TileContext(nc) as tc, ExitStack() as ctx:
        # SBUF pools
        xt_p = ctx.enter_context(tc.tile_pool(name="xt", bufs=1))
        wsm_p = ctx.enter_context(tc.tile_pool(name="wsm", bufs=2))
        wb_p = ctx.enter_context(tc.tile_pool(name="wb", bufs=2))
        rot_p = ctx.enter_context(tc.tile_pool(name="rot", bufs=2))
        vst_p = ctx.enter_context(tc.tile_pool(name="vst", bufs=1))
        pt_p = ctx.enter_context(tc.tile_pool(name="pt", bufs=2))
        qtb_p = ctx.enter_context(tc.tile_pool(name="qtb", bufs=1))
        tt_p = ctx.enter_context(tc.tile_pool(name="tt", bufs=1))
        nrc_p = ctx.enter_context(tc.tile_pool(name="nrc", bufs=1))
        attn_p = ctx.enter_context(tc.tile_pool(name="attn", bufs=1))
        cst_p = ctx.enter_context(tc.tile_pool(name="cst", bufs=1))
        ost_p = ctx.enter_context(tc.tile_pool(name="ost", bufs=4))
        # PSUM pools: pv 2x[128,1024] (4 banks) + sc [128,1024] (2 banks)
        # + flex 2 banks = 8
        flex_p = ctx.enter_context(tc.tile_pool(name="flex", bufs=2, space="PSUM"))
        sc_p = ctx.enter_context(tc.tile_pool(name="sc", bufs=1, space="PSUM"))
        pv_p = ctx.enter_context(tc.tile_pool(name="pv", bufs=1, space="PSUM"))

        # ---- DMA priority order: pair-0 proj weights + trig first, so the
        # first matmuls aren't stuck behind the bulk x transfer ----
        wsm0 = {}
        for which, w_d in (("q", wq_d), ("k", wk_d)):
            wsm = wsm_p.tile([128, 8, 128], bf16, tag=f"wsm{which}", name=f"w{which}0")
            _src = w_d[:, 0:128].rearrange("(a p) m -> p a m", p=128)
            for _i in range(4):
                nc.sync.dma_start(
                    wsm[:, 2 * _i : 2 * _i + 2, :], _src[:, 2 * _i : 2 * _i + 2, :]
                )
            wsm0[which] = wsm
        trig = cst_p.tile([128, 2 * S], bf16, tag="trig")
        for _i in range(2):
            nc.sync.dma_start(
                trig[:, _i * S : (_i + 1) * S], trig_d[:, _i * S : (_i + 1) * S]
            )
        cos_t = trig[:, 0:S]
        sin_t = trig[:, S : 2 * S]
        perm = cst_p.tile([128, 128], bf16, tag="perm")
        nc.sync.dma_start(perm[:], perm_d[:])

        # ---- xT resident (split transfers for queue parallelism) ----
        xt = []
        for k in range(8):
            t = xt_p.tile([128, S], bf16, tag=f"xt{k}", name=f"xt{k}")
            for _i in range(2):
                nc.sync.dma_start(
                    t[:, _i * 512 : (_i + 1) * 512],
                    xT_d[k * 128 : (k + 1) * 128, _i * 512 : (_i + 1) * 512],
                )
            xt.append(t)

        # ---- remaining constants ----
        bqc = cst_p.tile([128, 8], f32, tag="bqc")
        nc.sync.dma_start(bqc[:], bqc_d[:])
        bkc = cst_p.tile([128, 8], f32, tag="bkc")
        nc.sync.dma_start(bkc[:], bkc_d[:])
        bvb = cst_p.tile([128, HID], f32, tag="bvb")
        for _i in range(2):
            nc.sync.dma_start(
                bvb[:, _i * 512 : (_i + 1) * 512], bvb_d[:, _i * 512 : (_i + 1) * 512]
            )
        bob = cst_p.tile([128, HID], f32, tag="bob")
        for _i in range(2):
            nc.sync.dma_start(
                bob[:, _i * 512 : (_i + 1) * 512], bob_d[:, _i * 512 : (_i + 1) * 512]
            )

        # ---- per head pair: Q/K proj + rope as cost-annotated chunks so the
        # attention loop can pull ~uniform slices of PE work between score
        # emissions (keeps the PE stream dense and the exp WAR slack covered)
        def proj_rope_chunks(t, rots):
            """Returns a list of (pe_cost_ns, closure). Weight DMAs and tile
            allocations happen immediately; matmul/vector emission is deferred
            to the closures."""
            chunks = []
            for which, w_d, bcol in (("q", wq_d, bqc), ("k", wk_d, bkc)):
                if t == 0:
                    wsm = wsm0[which]
                else:
                    wsm = wsm_p.tile(
                        [128, 8, 128], bf16, tag=f"wsm{which}", name=f"w{which}{t}"
                    )
                    _src = w_d[:, t * 128 : (t + 1) * 128].rearrange(
                        "(a p) m -> p a m", p=128
                    )
                    for _i in range(4):
                        nc.sync.dma_start(
                            wsm[:, 2 * _i : 2 * _i + 2, :],
                            _src[:, 2 * _i : 2 * _i + 2, :],
                        )
                qtb = qtb_p.tile([128, S], bf16, tag="qtb", name=f"{which}tb{t}")
                rot = rot_p.tile(
                    [128, S], bf16, tag=f"rot{which}", name=f"{which}rot{t}"
                )

                def proj8(wsm, qtb, qc, which=which, bcol=bcol):
                    # full contraction chain in one chunk: splitting an open
                    # PSUM accumulation group across interleave points (other
                    # matmuls in between) measurably corrupts the partials
                    ps = flex_p.tile(
                        [128, 512], f32, tag="flex", name=f"{which}ps{t}_{qc}"
                    )
                    for k in range(8):
                        nc.tensor.matmul(
                            ps[:],
                            wsm[:, k, :],
                            xt[k][:, qc * 512 : (qc + 1) * 512],
                            start=(k == 0),
                            stop=(k == 7),
                        )
                    nc.vector.tensor_scalar(
                        qtb[:, qc * 512 : (qc + 1) * 512],
                        ps[:],
                        bcol[:, t : t + 1],
                        None,
                        op0=mybir.AluOpType.add,
                    )

                def ropec(qtb, rot, qc, which=which):
                    sl = slice(qc * 512, (qc + 1) * 512)
                    sw = flex_p.tile(
                        [128, 512], f32, tag="flex", name=f"{which}sw{t}_{qc}"
                    )
                    nc.tensor.matmul(sw[:], perm[:], qtb[:, sl], start=True, stop=True)
                    ta = tt_p.tile([128, 512], bf16, tag="ta", name=f"{which}ta{t}_{qc}")
                    nc.vector.tensor_mul(ta[:], qtb[:, sl], cos_t[:, sl])
                    tb = tt_p.tile([128, 512], bf16, tag="tb", name=f"{which}tb2{t}_{qc}")
                    nc.vector.tensor_mul(tb[:], sw[:], sin_t[:, sl])
                    nc.vector.tensor_add(rot[:, sl], ta[:], tb[:])

                for qc in range(2):
                    chunks.append(
                        (2060, (lambda f=proj8, w=wsm, q=qtb, c=qc: f(w, q, c)))
                    )
                for qc in range(2):
                    chunks.append(
                        (250, (lambda f=ropec, q=qtb, r=rot, c=qc: f(q, r, c)))
                    )
                rots[which] = rot
            return chunks

        def drain(chunks):
            for _, fn in chunks:
                fn()
            chunks.clear()

        def pull(chunks, budget_ns=650):
            pulled = 0
            while chunks and pulled < budget_ns:
                cost, fn = chunks.pop(0)
                fn()
                pulled += cost

        rots_cur = {}
        drain(proj_rope_chunks(0, rots_cur))

        # ---- V projection: natural [s, ones|dims], N=512 chunks, bias on
        # evict. Each head's stationary block is [64 ones | 64 dims] so the
        # augmented PV writes softmax denominators replicated across PSUM
        # partitions 0:64 (base-0: reciprocal_approx_fast drops nonzero base
        # partitions) and the numerators at 64:128 — no partition-shift or
        # broadcast needed later. ----
        vst = []
        for st in range(8):
            v = vst_p.tile([128, 16 * 128], bf16, tag=f"vst{st}", name=f"vst{st}")
            nc.vector.memset(v[:], 1.0)
            vst.append(v)
        for c2 in range(2):
            wb = wb_p.tile([128, 8, 512], bf16, tag="wb", name=f"wbv{c2}")
            _src = wv_d[:, c2 * 512 : (c2 + 1) * 512].rearrange("(a p) m -> p a m", p=128)
            for _i in range(4):
                nc.sync.dma_start(wb[:, 2 * _i : 2 * _i + 2, :], _src[:, 2 * _i : 2 * _i + 2, :])
            for st in range(8):
                ps = flex_p.tile([128, 512], f32, tag="flex", name=f"vps{c2}_{st}")
                for k in range(8):
                    nc.tensor.matmul(
                        ps[:],
                        xt[k][:, st * 128 : (st + 1) * 128],
                        wb[:, k, :],
                        start=(k == 0),
                        stop=(k == 7),
                    )
                nc.vector.tensor_tensor(
                    vst[st][:].rearrange("p (h c) -> p h c", c=128)[
                        :, 8 * c2 : 8 * c2 + 8, 64:128
                    ],
                    ps[:].rearrange("p (h c) -> p h c", c=64),
                    bvb[:, c2 * 512 : (c2 + 1) * 512].rearrange(
                        "p (h c) -> p h c", c=64
                    ),
                    op=mybir.AluOpType.add,
                )

        wbo_pre = {}
        next_rots = {}
        next_chunks = []
        attn = []
        for t in range(NPAIR):
            qrot, krot = rots_cur["q"], rots_cur["k"]
            if t + 1 < NPAIR:
                next_rots = {}
                next_chunks = proj_rope_chunks(t + 1, next_rots)
            else:
                next_chunks = []

            # -- scores + exp + augmented PV, streaming over k-tiles --
            pvt = [
                pv_p.tile([128, S], f32, tag="pvh0", name=f"pv{t}_0"),
                pv_p.tile([128, S], f32, tag="pvh1", name=f"pv{t}_1"),
            ]

            def emit_scores_qc(kt, qc, phh):
                """Both heads' score matmuls for one q-chunk, adjacent with
                row-split tile positions, then one fused exp over the pair."""
                ksl = slice(kt * 128, (kt + 1) * 128)
                qsl = slice(qc * 512, (qc + 1) * 512)
                scb = sc_p.tile([128, 2 * 512], f32, tag="scb", name=f"sc{t}_{kt}_{qc}")
                for h, (pr, tp) in enumerate(
                    ((slice(0, 64), (0, 0)), (slice(64, 128), (64, 0)))
                ):
                    nc.tensor.matmul(
                        scb[:, h * 512 : (h + 1) * 512],
                        krot[pr, ksl],
                        qrot[pr, qsl],
                        start=True,
                        stop=True,
                        tile_position=tp,
                    )
                # one exp over both heads: [128, 2, 512]
                nc.scalar.activation(
                    phh.rearrange("p (h q) -> p h q", h=2)[:, :, qsl],
                    scb[:].rearrange("p (h q) -> p h q", h=2),
                    EXP,
                    scale=0.125,
                )

            def emit_pv_qc(kt, qc, phh):
                qsl = slice(qc * 512, (qc + 1) * 512)
                phv = phh.rearrange("p (h q) -> p h q", h=2)
                for h in range(2):
                    vsl = slice((2 * t + h) * 128, (2 * t + h) * 128 + 128)
                    nc.tensor.matmul(
                        pvt[h][:, qsl],
                        vst[kt][:, vsl],
                        phv[:, h, qsl],
                        start=(kt == 0),
                        stop=(kt == 7),
                    )

            if t == NPAIR - 1:
                # prefetch first out-projection weight chunk
                wbo = wb_p.tile([128, 8, 512], bf16, tag="wb", name="wbo0")
                _src = wo_d[:, 0:512].rearrange("(a p) m -> p a m", p=128)
                for _i in range(4):
                    nc.sync.dma_start(
                        wbo[:, 2 * _i : 2 * _i + 2, :], _src[:, 2 * _i : 2 * _i + 2, :]
                    )
                wbo_pre[0] = wbo

            prev_ph = None
            for kt in range(8):
                phh = pt_p.tile([128, 2 * S], bf16, tag="phh", name=f"ph{t}_{kt}")
                for qc in range(2):
                    emit_scores_qc(kt, qc, phh)
                    if prev_ph is not None:
                        emit_pv_qc(kt - 1, qc, prev_ph)
                    pull(next_chunks)
                prev_ph = phh
            for qc in range(2):
                emit_pv_qc(7, qc, prev_ph)
            drain(next_chunks)

            # -- normalization: denominators sit replicated in pvt rows
            #    0:64 (ones-block aug) -> reciprocal -> multiply
            at = attn_p.tile([128, S], bf16, tag=f"attn{t}", name=f"attn{t}")
            for h in range(2):
                nr = nrc_p.tile([64, S], f32, tag=f"nrc{h}", name=f"nr{t}_{h}")
                nc.vector.reciprocal_approx_fast(out=nr[:], in_=pvt[h][0:64, :])
                # attnT rows h*64..h*64+64  <-  out_un rows 64:128 of pvt[h]
                nc.vector.tensor_mul(
                    at[h * 64 : h * 64 + 64, :], pvt[h][64:128, :], nr[:]
                )
            attn.append(at)
            rots_cur = next_rots

        # ---- output projection: N=512 chunks, bias on evict ----
        for c2 in range(2):
            if c2 in wbo_pre:
                wb = wbo_pre[c2]
            else:
                wb = wb_p.tile([128, 8, 512], bf16, tag="wb", name=f"wbo{c2}")
                _src = wo_d[:, c2 * 512 : (c2 + 1) * 512].rearrange(
                    "(a p) m -> p a m", p=128
                )
                for _i in range(4):
                    nc.sync.dma_start(
                        wb[:, 2 * _i : 2 * _i + 2, :], _src[:, 2 * _i : 2 * _i + 2, :]
                    )
            for qt in range(8):
                ps = flex_p.tile([128, 512], f32, tag="flex", name=f"ops{c2}_{qt}")
                for dt in range(8):
                    nc.tensor.matmul(
                        ps[:],
                        attn[dt][:, qt * 128 : (qt + 1) * 128],
                        wb[:, dt, :],
                        start=(dt == 0),
                        stop=(dt == 7),
                    )
                ostg = ost_p.tile([128, 512], f32, tag="ostg", name=f"ostg{c2}_{qt}")
                nc.vector.tensor_tensor(
                    ostg[:],
                    ps[:],
                    bob[:, c2 * 512 : (c2 + 1) * 512],
                    op=mybir.AluOpType.add,
                )
                for _i in range(2):
                    nc.sync.dma_start(
                        out_d[
                            qt * 128 : (qt + 1) * 128,
                            c2 * 512 + _i * 256 : c2 * 512 + (_i + 1) * 256,
                        ],
                        ostg[:, _i * 256 : (_i + 1) * 256],
                    )

    nc.compile()
    return nc


def _trig_tables(height, width):
    """COS / SIN' tables in [dim, s] pair-tile layout, fp32 arithmetic to match
    the reference's fp32 angle computation, cast to bf16 at the end."""
    s = np.arange(S, dtype=np.float32)
    ang = np.zeros((64, S), dtype=np.float32)
    inv1 = (
        1.0 / (BASE ** (np.arange(0, LAT, 2, dtype=np.float32) / np.float32(LAT)))
    ).astype(np.float32)
    half = 24
    inv2 = (
        1.0 / (BASE ** (np.arange(0, half, 2, dtype=np.float32) / np.float32(half)))
    ).astype(np.float32)
    col = (np.arange(S) % width).astype(np.float32)
    row = (np.arange(S) // width).astype(np.float32)
    for j in range(8):
        a = (s * inv1[j]).astype(np.float32)
        ang[2 * j] = a
        ang[2 * j + 1] = a
    for j in range(12):
        a = (col * inv2[j]).astype(np.float32)
        ang[16 + 2 * j] = a
        ang[16 + 2 * j + 1] = a
        b = (row * inv2[j]).astype(np.float32)
        ang[40 + 2 * j] = b
        ang[40 + 2 * j + 1] = b
    cos64 = np.cos(ang).astype(np.float32)
    sin64 = np.sin(ang).astype(np.float32)
    sgn = np.tile(np.array([[-1.0], [1.0]], np.float32), (32, 1))
    sinp = (sin64 * sgn).astype(np.float32)
    cos128 = np.concatenate([cos64, cos64], axis=0)
    sinp128 = np.concatenate([sinp, sinp], axis=0)
    return np.concatenate([cos128, sinp128], axis=1).astype(np_bf16)


def _static_consts(bq, bk, bv, bo):
    permm = np.zeros((128, 128), np.float32)
    for k in range(128):
        partner = k + 1 if k % 2 == 0 else k - 1
        permm[k, partner] = 1.0
    return {
        "perm": permm.astype(np_bf16),
        "bqc": np.ascontiguousarray(bq.reshape(8, 128).T).astype(np.float32),
        "bkc": np.ascontiguousarray(bk.reshape(8, 128).T).astype(np.float32),
        "bvb": np.broadcast_to(bv.reshape(1, HID), (128, HID)).astype(np.float32),
        "bob": np.broadcast_to(bo.reshape(1, HID), (128, HID)).astype(np.float32),
    }


def _run(inputs, trace):
    x = np.asarray(inputs["x"], np.float32)
    Wq = np.asarray(inputs["Wq"], np.float32).astype(np_bf16)
    Wk = np.asarray(inputs["Wk"], np.float32).astype(np_bf16)
    Wv = np.asarray(inputs["Wv"], np.float32).astype(np_bf16)
    Wo = np.asarray(inputs["Wo"], np.float32).astype(np_bf16)
    bq = np.asarray(inputs["bq"], np.float32)
    bk = np.asarray(inputs["bk"], np.float32)
    bv = np.asarray(inputs["bv"], np.float32)
    bo = np.asarray(inputs["bo"], np.float32)
    height = int(inputs["height"])
    width = int(inputs["width"])
    B = x.shape[0]
    assert B == N_CORES and x.shape[1] == S and x.shape[2] == HID

    if "nc" not in _CACHE:
        _CACHE["nc"] = _build_nc()
    nc = _CACHE["nc"]

    trig = _trig_tables(height, width)
    consts = _static_consts(bq, bk, bv, bo)
    in_maps = []
    for c in range(N_CORES):
        m = {
            "xT": np.ascontiguousarray(x[c].T).astype(np_bf16),
            "wq": Wq,
            "wk": Wk,
            "wv": Wv,
            "wo": Wo,
            "bqc": consts["bqc"],
            "bkc": consts["bkc"],
            "bvb": consts["bvb"],
            "bob": consts["bob"],
            "trig": trig,
            "perm": consts["perm"],
        }
        in_maps.append(m)
    res = run_bass_kernel_spmd(nc, in_maps, list(range(N_CORES)), trace=trace)
    y = np.stack([res.results[c]["out"] for c in range(N_CORES)], axis=0)
    return y.astype(np.float32), res.exec_time_ns


def kernel(**inputs):
    y, _ = _run(inputs, trace=False)
    return y


def kernel_profiled(**inputs):
    """Like kernel() but also returns NTFF-profiled HW exec time (ns)."""
    return _run(inputs, trace=True)


# revision 3
# speedup vs baseline: 1.0091x; 1.0091x over previous
"""Trainium2 Bass kernel for nn_Attention_40149354283630 — v6.

Multi-head attention (16 heads, head_dim 64) with mixed 1D-latent + axial-2D
spatial RoPE, over x:(8, 1024, 1024). Data-parallel over the batch dim across
8 NeuronCores; each core runs the full transformer block for one batch element.

v2 changes vs baseline (439us):
  - all matmul operands bf16 (fp32 PSUM accumulation). Halves LDWEIGHTS time
    (which rate-limits back-to-back matmuls on TRN2 since every fp32r matmul
    self-loads its stationary), halves SBUF traffic, halves weight DMA.
  - score matmuls for the head pair emitted adjacently with row-split
    tile_position (0,0)/(64,0) so the two C=64 matmuls can overlap in the
    PE array.
  - C=1 bias-seed matmuls eliminated (V / out projections): bias rows are
    pre-broadcast to [128, 1024] tiles on the host and fused into the PSUM
    eviction as tensor_tensor adds.
  - V / out projections use N=512 moving chunks (half the matmul + LDW count).
  - exp fused across the head pair: one activation [128, 2x512] per (kt, qc)
    reading a 2-bank PSUM score tile, writing bf16 P^T.
  - dedicated PSUM pools: pv 2x[65,1024] (4 banks) + sc [128,1024] (2 banks)
    + flex 2 banks, so attention-phase tiles never rotate through the same
    banks as the interleaved projection chains.

Per-core dataflow:
  xT [hid, s] bf16 (host-transposed/cast)
  V  = (xT.T @ Wv + bvb)           natural [s, dims], bias added on evict
  QT = (Wq.T @ xT + bq)            transposed [dims, s], bias via tensor_scalar
  rope: QTrot = QT*COS + (PERM@QT)*SIN'   (pair-swap via permutation matmul)
  per head-pair t (2 heads per 128-partition tile):
    S^T[k,q] = KTrot.T @ QTrot     row-split tile_position, heads adjacent
    P^T      = exp(S^T / 8)        one ScalarE instr per (kt,qc), bf16 out
    [outT;sums] = [V|1].T @ P^T    M=65 augmented PV, sums ride in row 64
    nrm      = 1/(ones64 outer sums)  row-broadcast matmul + reciprocal
    attnT    = outT * nrm          partition-shifted write packs the head pair
  out = attnT.T @ Wo + bob         natural [s, hid], bias added on evict
"""

import contextlib
import ctypes
import sys
import types

import numpy as np
from contextlib import ExitStack

import ml_dtypes

import concourse.bass as bass
import concourse.tile as tile
from concourse import bacc, mybir
from concourse.bass_utils import run_bass_kernel_spmd


def _install_ntff_hook_shim():
    """Provide antenv.axon_hooks if the image lacks it, so that
    run_bass_kernel_spmd(trace=True) (or BASS_TRACE=1) profiles instead of
    crashing. Mirrors trn_agent_boot's ctypes hook against libaxon_pjrt.so."""
    try:
        from antenv.axon_hooks import get_axon_ntff_profile_hook  # noqa: F401

        return
    except ImportError:
        pass
    try:
        import antenv
    except ImportError:
        return
    store = {"hook": None}
    so_path = "/opt/axon/libaxon_pjrt.so"
    try:
        lib = ctypes.CDLL(so_path)
        if hasattr(lib, "axon_start_nrt_profile"):
            lib.axon_start_nrt_profile.argtypes = [
                ctypes.POINTER(ctypes.c_int64),
                ctypes.c_size_t,
            ]
            lib.axon_start_nrt_profile.restype = ctypes.c_int64
            lib.axon_stop_nrt_profile.argtypes = [ctypes.c_char_p]
            lib.axon_stop_nrt_profile.restype = ctypes.c_int64

            @contextlib.contextmanager
            def _hook(output_dir, device_ids):
                import jax

                jax.devices()
                if device_ids:
                    ids = (ctypes.c_int64 * len(device_ids))(*device_ids)
                    rc = lib.axon_start_nrt_profile(ids, len(device_ids))
                else:
                    rc = lib.axon_start_nrt_profile(None, 0)
                if rc != 0:
                    raise RuntimeError(f"axon_start_nrt_profile rc={rc}")
                try:
                    yield
                finally:
                    n = lib.axon_stop_nrt_profile(str(output_dir).encode())
                    if n < 0:
                        raise RuntimeError(f"axon_stop_nrt_profile rc={n}")

            store["hook"] = _hook
    except OSError:
        pass
    mod = types.ModuleType("antenv.axon_hooks")
    mod.get_axon_ntff_profile_hook = lambda: store["hook"]
    mod.set_axon_ntff_profile_hook = lambda h: store.__setitem__("hook", h)
    sys.modules["antenv.axon_hooks"] = mod
    antenv.axon_hooks = mod


_install_ntff_hook_shim()

N_CORES = 8
HID, NH, HD = 1024, 16, 64
S = 1024
LAT, BASE = 16, 10000.0
NPAIR = 8  # head-pair tiles (2 heads x 64 dims = 128 partitions)

f32 = mybir.dt.float32
f32r = mybir.dt.float32r
bf16 = mybir.dt.bfloat16
np_bf16 = ml_dtypes.bfloat16
EXP = mybir.ActivationFunctionType.Exp

_CACHE = {}


def _build_nc():
    nc = bacc.Bacc("TRN2", target_bir_lowering=False, debug=False, num_devices=N_CORES)

    def din(name, shape, dt):
        return nc.dram_tensor(name, shape, dt, kind="ExternalInput").ap()

    xT_d = din("xT", [HID, S], bf16)
    wq_d = din("wq", [HID, HID], bf16)
    wk_d = din("wk", [HID, HID], bf16)
    wv_d = din("wv", [HID, HID], bf16)
    wo_d = din("wo", [HID, HID], bf16)
    bqc_d = din("bqc", [128, 8], f32)
    bkc_d = din("bkc", [128, 8], f32)
    bvb_d = din("bvb", [128, HID], f32)  # bv broadcast over partitions
    bob_d = din("bob", [128, HID], f32)  # bo broadcast over partitions
    trig_d = din("trig", [128, 2 * S], bf16)  # cols 0:S = COS, S:2S = SIN'
    perm_d = din("perm", [128, 128], bf16)
    out_d = nc.dram_tensor("out", [S, HID], f32, kind="ExternalOutput").ap()

    with tile.TileContext(nc) as tc, ExitStack() as ctx:
        # SBUF pools
        xt_p = ctx.enter_context(tc.tile_pool(name="xt", bufs=1))
        wsm_p = ctx.enter_context(tc.tile_pool(name="wsm", bufs=2))
        wb_p = ctx.enter_context(tc.tile_pool(name="wb", bufs=2))
        rot_p = ctx.enter_context(tc.tile_pool(name="rot", bufs=2))
        vst_p = ctx.enter_context(tc.tile_pool(name="vst", bufs=1))
        pt_p = ctx.enter_context(tc.tile_pool(name="pt", bufs=2))
        qtb_p = ctx.enter_context(tc.tile_pool(name="qtb", bufs=1))
        tt_p = ctx.enter_context(tc.tile_pool(name="tt", bufs=1))
        nrc_p = ctx.enter_context(tc.tile_pool(name="nrc", bufs=1))
        attn_p = ctx.enter_context(tc.tile_pool(name="attn", bufs=1))
        cst_p = ctx.enter_context(tc.tile_pool(name="cst", bufs=1))
        ost_p = ctx.enter_context(tc.tile_pool(name="ost", bufs=4))
        # PSUM pools: pv 2x[128,512] (2 banks) + double-buffered sc
        # 2x[128,1024] (4 banks) + flex 2 banks = 8
        flex_p = ctx.enter_context(tc.tile_pool(name="flex", bufs=2, space="PSUM"))
        sc_p = ctx.enter_context(tc.tile_pool(name="sc", bufs=2, space="PSUM"))
        pv_p = ctx.enter_context(tc.tile_pool(name="pv", bufs=1, space="PSUM"))

        # ---- DMA priority order: pair-0 proj weights + trig first, so the
        # first matmuls aren't stuck behind the bulk x transfer ----
        wsm0 = {}
        for which, w_d in (("q", wq_d), ("k", wk_d)):
            wsm = wsm_p.tile([128, 8, 128], bf16, tag=f"wsm{which}", name=f"w{which}0")
            _src = w_d[:, 0:128].rearrange("(a p) m -> p a m", p=128)
            for _i in range(4):
                nc.sync.dma_start(
                    wsm[:, 2 * _i : 2 * _i + 2, :], _src[:, 2 * _i : 2 * _i + 2, :]
                )
            wsm0[which] = wsm
        trig = cst_p.tile([128, 2 * S], bf16, tag="trig")
        for _i in range(2):
            nc.sync.dma_start(
                trig[:, _i * S : (_i + 1) * S], trig_d[:, _i * S : (_i + 1) * S]
            )
        cos_t = trig[:, 0:S]
        sin_t = trig[:, S : 2 * S]
        perm = cst_p.tile([128, 128], bf16, tag="perm")
        nc.sync.dma_start(perm[:], perm_d[:])

        # ---- xT resident (split transfers for queue parallelism) ----
        xt = []
        for k in range(8):
            t = xt_p.tile([128, S], bf16, tag=f"xt{k}", name=f"xt{k}")
            for _i in range(2):
                nc.sync.dma_start(
                    t[:, _i * 512 : (_i + 1) * 512],
                    xT_d[k * 128 : (k + 1) * 128, _i * 512 : (_i + 1) * 512],
                )
            xt.append(t)

        # ---- remaining constants ----
        bqc = cst_p.tile([128, 8], f32, tag="bqc")
        nc.sync.dma_start(bqc[:], bqc_d[:])
        bkc = cst_p.tile([128, 8], f32, tag="bkc")
        nc.sync.dma_start(bkc[:], bkc_d[:])
        bvb = cst_p.tile([128, HID], f32, tag="bvb")
        for _i in range(2):
            nc.sync.dma_start(
                bvb[:, _i * 512 : (_i + 1) * 512], bvb_d[:, _i * 512 : (_i + 1) * 512]
            )
        bob = cst_p.tile([128, HID], f32, tag="bob")
        for _i in range(2):
            nc.sync.dma_start(
                bob[:, _i * 512 : (_i + 1) * 512], bob_d[:, _i * 512 : (_i + 1) * 512]
            )

        # ---- per head pair: Q/K proj + rope as cost-annotated chunks so the
        # attention loop can pull ~uniform slices of PE work between score
        # emissions (keeps the PE stream dense and the exp WAR slack covered)
        def proj_rope_chunks(t, rots):
            """Returns a list of (pe_cost_ns, closure). Weight DMAs and tile
            allocations happen immediately; matmul/vector emission is deferred
            to the closures."""
            chunks = []
            for which, w_d, bcol in (("q", wq_d, bqc), ("k", wk_d, bkc)):
                if t == 0:
                    wsm = wsm0[which]
                else:
                    wsm = wsm_p.tile(
                        [128, 8, 128], bf16, tag=f"wsm{which}", name=f"w{which}{t}"
                    )
                    _src = w_d[:, t * 128 : (t + 1) * 128].rearrange(
                        "(a p) m -> p a m", p=128
                    )
                    for _i in range(4):
                        nc.sync.dma_start(
                            wsm[:, 2 * _i : 2 * _i + 2, :],
                            _src[:, 2 * _i : 2 * _i + 2, :],
                        )
                qtb = qtb_p.tile([128, S], bf16, tag="qtb", name=f"{which}tb{t}")
                rot = rot_p.tile(
                    [128, S], bf16, tag=f"rot{which}", name=f"{which}rot{t}"
                )

                def proj8(wsm, qtb, qc, which=which, bcol=bcol):
                    # full contraction chain in one chunk: splitting an open
                    # PSUM accumulation group across interleave points (other
                    # matmuls in between) measurably corrupts the partials
                    ps = flex_p.tile(
                        [128, 512], f32, tag="flex", name=f"{which}ps{t}_{qc}"
                    )
                    for k in range(8):
                        nc.tensor.matmul(
                            ps[:],
                            wsm[:, k, :],
                            xt[k][:, qc * 512 : (qc + 1) * 512],
                            start=(k == 0),
                            stop=(k == 7),
                        )
                    nc.vector.tensor_scalar(
                        qtb[:, qc * 512 : (qc + 1) * 512],
                        ps[:],
                        bcol[:, t : t + 1],
                        None,
                        op0=mybir.AluOpType.add,
                    )

                def ropec(qtb, rot, qc, which=which):
                    sl = slice(qc * 512, (qc + 1) * 512)
                    sw = flex_p.tile(
                        [128, 512], f32, tag="flex", name=f"{which}sw{t}_{qc}"
                    )
                    nc.tensor.matmul(sw[:], perm[:], qtb[:, sl], start=True, stop=True)
                    ta = tt_p.tile([128, 512], bf16, tag="ta", name=f"{which}ta{t}_{qc}")
                    nc.vector.tensor_mul(ta[:], qtb[:, sl], cos_t[:, sl])
                    tb = tt_p.tile([128, 512], bf16, tag="tb", name=f"{which}tb2{t}_{qc}")
                    nc.vector.tensor_mul(tb[:], sw[:], sin_t[:, sl])
                    nc.vector.tensor_add(rot[:, sl], ta[:], tb[:])

                for qc in range(2):
                    chunks.append(
                        (2060, (lambda f=proj8, w=wsm, q=qtb, c=qc: f(w, q, c)))
                    )
                for qc in range(2):
                    chunks.append(
                        (250, (lambda f=ropec, q=qtb, r=rot, c=qc: f(q, r, c)))
                    )
                rots[which] = rot
            return chunks

        def drain(chunks):
            for _, fn in chunks:
                fn()
            chunks.clear()

        def pull(chunks, budget_ns=650):
            pulled = 0
            while chunks and pulled < budget_ns:
                cost, fn = chunks.pop(0)
                fn()
                pulled += cost

        rots_cur = {}
        drain(proj_rope_chunks(0, rots_cur))

        # ---- V projection: natural [s, ones|dims], N=512 chunks, bias on
        # evict. Each head's stationary block is [64 ones | 64 dims] so the
        # augmented PV writes softmax denominators replicated across PSUM
        # partitions 0:64 (base-0: reciprocal_approx_fast drops nonzero base
        # partitions) and the numerators at 64:128 — no partition-shift or
        # broadcast needed later. ----
        vst = []
        for st in range(8):
            v = vst_p.tile([128, 16 * 128], bf16, tag=f"vst{st}", name=f"vst{st}")
            nc.vector.memset(v[:], 1.0)
            vst.append(v)
        for c2 in range(2):
            wb = wb_p.tile([128, 8, 512], bf16, tag="wb", name=f"wbv{c2}")
            _src = wv_d[:, c2 * 512 : (c2 + 1) * 512].rearrange("(a p) m -> p a m", p=128)
            for _i in range(4):
                nc.sync.dma_start(wb[:, 2 * _i : 2 * _i + 2, :], _src[:, 2 * _i : 2 * _i + 2, :])
            for st in range(8):
                ps = flex_p.tile([128, 512], f32, tag="flex", name=f"vps{c2}_{st}")
                for k in range(8):
                    nc.tensor.matmul(
                        ps[:],
                        xt[k][:, st * 128 : (st + 1) * 128],
                        wb[:, k, :],
                        start=(k == 0),
                        stop=(k == 7),
                    )
                nc.vector.tensor_tensor(
                    vst[st][:].rearrange("p (h c) -> p h c", c=128)[
                        :, 8 * c2 : 8 * c2 + 8, 64:128
                    ],
                    ps[:].rearrange("p (h c) -> p h c", c=64),
                    bvb[:, c2 * 512 : (c2 + 1) * 512].rearrange(
                        "p (h c) -> p h c", c=64
                    ),
                    op=mybir.AluOpType.add,
                )

        wbo_pre = {}
        next_rots = {}
        next_chunks = []
        attn = []
        for t in range(NPAIR):
            qrot, krot = rots_cur["q"], rots_cur["k"]
            if t + 1 < NPAIR:
                next_rots = {}
                next_chunks = proj_rope_chunks(t + 1, next_rots)
            else:
                next_chunks = []

            # -- per q-half sweep: scores + fused exp + augmented PV over
            # all k-tiles. PV accumulates in single-bank PSUM tiles; score
            # tiles double-buffer (4 banks) so the next score matmul's
            # bank-reuse WAR resolves two exps back instead of one --
            at = attn_p.tile([128, S], bf16, tag=f"attn{t}", name=f"attn{t}")

            if t == NPAIR - 1:
                # prefetch first out-projection weight chunk
                wbo = wb_p.tile([128, 8, 512], bf16, tag="wb", name="wbo0")
                _src = wo_d[:, 0:512].rearrange("(a p) m -> p a m", p=128)
                for _i in range(4):
                    nc.sync.dma_start(
                        wbo[:, 2 * _i : 2 * _i + 2, :], _src[:, 2 * _i : 2 * _i + 2, :]
                    )
                wbo_pre[0] = wbo

            for qc in range(2):
                qsl = slice(qc * 512, (qc + 1) * 512)
                pvt = [
                    pv_p.tile([128, 512], f32, tag=f"pvh{h}", name=f"pv{t}_{qc}_{h}")
                    for h in range(2)
                ]

                def emit_scores(kt, phh, qc=qc, qsl=qsl):
                    ksl = slice(kt * 128, (kt + 1) * 128)
                    scb = sc_p.tile(
                        [128, 2 * 512], f32, tag="scb", name=f"sc{t}_{qc}_{kt}"
                    )
                    for h, (pr, tp) in enumerate(
                        ((slice(0, 64), (0, 0)), (slice(64, 128), (64, 0)))
                    ):
                        nc.tensor.matmul(
                            scb[:, h * 512 : (h + 1) * 512],
                            krot[pr, ksl],
                            qrot[pr, qsl],
                            start=True,
                            stop=True,
                            tile_position=tp,
                        )
                    # one exp over both heads, 2D contiguous in and out
                    nc.scalar.activation(phh[:], scb[:], EXP, scale=0.125)

                def emit_pv(kt, phh, pvt=pvt):
                    for h in range(2):
                        vsl = slice((2 * t + h) * 128, (2 * t + h) * 128 + 128)
                        nc.tensor.matmul(
                            pvt[h][:],
                            vst[kt][:, vsl],
                            phh[:, h * 512 : (h + 1) * 512],
                            start=(kt == 0),
                            stop=(kt == 7),
                        )

                prev_ph = None
                for kt in range(8):
                    phh = pt_p.tile(
                        [128, 2 * 512], bf16, tag="phh", name=f"ph{t}_{qc}_{kt}"
                    )
                    emit_scores(kt, phh)
                    if prev_ph is not None:
                        emit_pv(kt - 1, prev_ph)
                    pull(next_chunks)
                    prev_ph = phh
                emit_pv(7, prev_ph)

                # -- normalization for this q-half: denominators replicated in
                # pvt rows 0:64 (ones-block aug) -> reciprocal -> multiply
                for h in range(2):
                    nr = nrc_p.tile(
                        [64, 512], f32, tag=f"nrc{h}", name=f"nr{t}_{qc}_{h}"
                    )
                    nc.vector.reciprocal_approx_fast(out=nr[:], in_=pvt[h][0:64, :])
                    nc.vector.tensor_mul(
                        at[h * 64 : h * 64 + 64, qsl], pvt[h][64:128, :], nr[:]
                    )
            drain(next_chunks)
            attn.append(at)
            rots_cur = next_rots

        # ---- output projection: N=512 chunks, bias on evict ----
        for c2 in range(2):
            if c2 in wbo_pre:
                wb = wbo_pre[c2]
            else:
                wb = wb_p.tile([128, 8, 512], bf16, tag="wb", name=f"wbo{c2}")
                _src = wo_d[:, c2 * 512 : (c2 + 1) * 512].rearrange(
                    "(a p) m -> p a m", p=128
                )
                for _i in range(4):
                    nc.sync.dma_start(
                        wb[:, 2 * _i : 2 * _i + 2, :], _src[:, 2 * _i : 2 * _i + 2, :]
                    )
            for qt in range(8):
                ps = flex_p.tile([128, 512], f32, tag="flex", name=f"ops{c2}_{qt}")
                for dt in range(8):
                    nc.tensor.matmul(
                        ps[:],
                        attn[dt][:, qt * 128 : (qt + 1) * 128],
                        wb[:, dt, :],
                        start=(dt == 0),
                        stop=(dt == 7),
                    )
                ostg = ost_p.tile([128, 512], f32, tag="ostg", name=f"ostg{c2}_{qt}")
                nc.vector.tensor_tensor(
                    ostg[:],
                    ps[:],
                    bob[:, c2 * 512 : (c2 + 1) * 512],
                    op=mybir.AluOpType.add,
                )
                for _i in range(2):
                    nc.sync.dma_start(
                        out_d[
                            qt * 128 : (qt + 1) * 128,
                            c2 * 512 + _i * 256 : c2 * 512 + (_i + 1) * 256,
                        ],
                        ostg[:, _i * 256 : (_i + 1) * 256],
                    )

    nc.compile()
    return nc


def _trig_tables(height, width):
    """COS / SIN' tables in [dim, s] pair-tile layout, fp32 arithmetic to match
    the reference's fp32 angle computation, cast to bf16 at the end."""
    s = np.arange(S, dtype=np.float32)
    ang = np.zeros((64, S), dtype=np.float32)
    inv1 = (
        1.0 / (BASE ** (np.arange(0, LAT, 2, dtype=np.float32) / np.float32(LAT)))
    ).astype(np.float32)
    half = 24
    inv2 = (
        1.0 / (BASE ** (np.arange(0, half, 2, dtype=np.float32) / np.float32(half)))
    ).astype(np.float32)
    col = (np.arange(S) % width).astype(np.float32)
    row = (np.arange(S) // width).astype(np.float32)
    for j in range(8):
        a = (s * inv1[j]).astype(np.float32)
        ang[2 * j] = a
        ang[2 * j + 1] = a
    for j in range(12):
        a = (col * inv2[j]).astype(np.float32)
        ang[16 + 2 * j] = a
        ang[16 + 2 * j + 1] = a
        b = (row * inv2[j]).astype(np.float32)
        ang[40 + 2 * j] = b
        ang[40 + 2 * j + 1] = b
    cos64 = np.cos(ang).astype(np.float32)
    sin64 = np.sin(ang).astype(np.float32)
    sgn = np.tile(np.array([[-1.0], [1.0]], np.float32), (32, 1))
    sinp = (sin64 * sgn).astype(np.float32)
    cos128 = np.concatenate([cos64, cos64], axis=0)
    sinp128 = np.concatenate([sinp, sinp], axis=0)
    return np.concatenate([cos128, sinp128], axis=1).astype(np_bf16)


def _static_consts(bq, bk, bv, bo):
    permm = np.zeros((128, 128), np.float32)
    for k in range(128):
        partner = k + 1 if k % 2 == 0 else k - 1
        permm[k, partner] = 1.0
    return {
        "perm": permm.astype(np_bf16),
        "bqc": np.ascontiguousarray(bq.reshape(8, 128).T).astype(np.float32),
        "bkc": np.ascontiguousarray(bk.reshape(8, 128).T).astype(np.float32),
        "bvb": np.broadcast_to(bv.reshape(1, HID), (128, HID)).astype(np.float32),
        "bob": np.broadcast_to(bo.reshape(1, HID), (128, HID)).astype(np.float32),
    }


def _run(inputs, trace):
    x = np.asarray(inputs["x"], np.float32)
    Wq = np.asarray(inputs["Wq"], np.float32).astype(np_bf16)
    Wk = np.asarray(inputs["Wk"], np.float32).astype(np_bf16)
    Wv = np.asarray(inputs["Wv"], np.float32).astype(np_bf16)
    Wo = np.asarray(inputs["Wo"], np.float32).astype(np_bf16)
    bq = np.asarray(inputs["bq"], np.float32)
    bk = np.asarray(inputs["bk"], np.float32)
    bv = np.asarray(inputs["bv"], np.float32)
    bo = np.asarray(inputs["bo"], np.float32)
    height = int(inputs["height"])
    width = int(inputs["width"])
    B = x.shape[0]
    assert B == N_CORES and x.shape[1] == S and x.shape[2] == HID

    if "nc" not in _CACHE:
        _CACHE["nc"] = _build_nc()
    nc = _CACHE["nc"]

    trig = _trig_tables(height, width)
    consts = _static_consts(bq, bk, bv, bo)
    in_maps = []
    for c in range(N_CORES):
        m = {
            "xT": np.ascontiguousarray(x[c].T).astype(np_bf16),
            "wq": Wq,
            "wk": Wk,
            "wv": Wv,
            "wo": Wo,
            "bqc": consts["bqc"],
            "bkc": consts["bkc"],
            "bvb": consts["bvb"],
            "bob": consts["bob"],
            "trig": trig,
            "perm": consts["perm"],
        }
        in_maps.append(m)
    res = run_bass_kernel_spmd(nc, in_maps, list(range(N_CORES)), trace=trace)
    y = np.stack([res.results[c]["out"] for c in range(N_CORES)], axis=0)
    return y.astype(np.float32), res.exec_time_ns


def kernel(**inputs):
    y, _ = _run(inputs, trace=False)
    return y


def kernel_profiled(**inputs):
    """Like kernel() but also returns NTFF-profiled HW exec time (ns)."""
    return _run(inputs, trace=True)
